# revision 3
# baseline (speedup 1.0000x reference)
"""DSNAS MoE-routing forward kernel for 8 Trainium2 NeuronCores (V2.4).

Computation (see reference): for each of 28 column pairs (i,j), with hard
top-1 routing l = argmax(log_alpha[k]):
    p = M[i] + S01[i]*noise[k,0],  q = M[j] + S01[j]*noise[k,1]
    out += branch_l(p, q) @ W_l.T
where M = emb_mean gathered by features, S01 = softplus(emb_std)*0.01.

Data-parallel over batch B=8192 -> 1024 rows/core; tables replicated.
Device layout [D=128 partitions, B free], all-bf16 math (PSUM fp32): the
noise term is 1e-2-scale so fp8/bf16 rounding there is ~1e-4 relative on
the output; bf16 on the mean path ~2e-3.  Gate is 2e-2.

Structure:
- noise ships fp8e4 [D, NPAIR*2*BS] in pair order, upconverted fp8->bf16
  in-flight by chunked gpsimd (SWDGE) casting DMAs into a RING of SBUF
  tiles (exact cast; halves HBM reads; keeps DVE in 2x mode).
- s/m tables gathered per column into two big SBUF tiles via one-hot
  matmuls; t0|t1 = s*(n0|n1) is ONE fused [D,2,BS] DVE op per pair using
  a two-block access pattern into the s table (stride (j-i)*BS).
- add/cat pairs: out += t0@Wp + t1@Wq; mean path via stacked per-column
  CM tables (oh96, K=96 matmul, hi+lo bf16).
- mul pairs: p|q = t + (m_i|m_j) fused add, combo = p*q, one matmul.
- max/min pairs: max(p,q) = p + relu(q-p): d accumulates in PSUM via a
  host-built stacked +-M table (one K=96 matmul) plus I@(t1-t0); relu on
  the Scalar engine; out += t0@W + r@(+-W).  p's mean rides CM.
- HAM discipline: the PE re-throttles to 1.2GHz after any ~3.4us idle
  window, and the PE queue is in-order, so each pair's accumulation
  matmuls are emitted LAG pairs late, in front of the next pair's
  stall-prone work -- the PE always has ready work ahead of a stall.
"""

import os
import sys
from collections import deque

import numpy as np
import ml_dtypes

for _p in ("/opt/trn_rl_repo",):
    if _p not in sys.path and os.path.isdir(_p):
        sys.path.insert(0, _p)

import concourse.bacc as bacc
import concourse.bass as bass
import concourse.mybir as mybir
import concourse.tile as tile
from concourse.bass_utils import run_bass_kernel_spmd

COLS = 8
D = 128
B = 8192
NUM_EMB = 12
PAIRS = [(i, j) for i in range(COLS) for j in range(COLS) if i < j]
NPAIR = len(PAIRS)  # 28
NCORES = 8
BS = B // NCORES  # 1024 per core
CH = 512  # matmul free-dim chunk (one PSUM bank of fp32)
NCH = BS // CH

FP32 = mybir.dt.float32
BF16 = mybir.dt.bfloat16
FP8 = mybir.dt.float8e4
BF = ml_dtypes.bfloat16
F8 = ml_dtypes.float8_e4m3

_ALU = [
    mybir.AluOpType.add,
    mybir.AluOpType.mult,
    mybir.AluOpType.max,
    mybir.AluOpType.min,
]

# knobs
NWARM = int(os.environ.get("KV_WARM", "30"))  # junk matmuls to burn the cold window
NRELU = int(os.environ.get("KV_NRELU", "10"))  # max/min pairs on the relu path
NZC = int(os.environ.get("KV_NZC", "2"))  # pairs per noise DMA/ring tile
NZ_FP8 = os.environ.get("KV_FP8", "1") == "1"  # fp8 wire + casting DMA
LAG = int(os.environ.get("KV_LAG", "2"))  # acc deferral (pairs) for PE reservoir
DN_DVE = os.environ.get("KV_DN", "1") == "1"  # t-diff on DVE vs PE I-matmuls

# fat gather layout: one-hots and tables packed at 32-row strips, 4
# columns per [128, *] tensor half (matmul tile_position allows row
# bases {0,32,64,96}), so the head DMAs are wide and fast

# oh96 (bf16, [COLS*NUM_EMB, BS + 4]): rows c*12+e = onehot col c; last 4
# cols hold stacked CM mean tables [hi(2) | lo(2)]
OHW = BS + 4


def _plan(pos):
    """Sort pairs, pick per-pair strategy, build the korder."""
    ksort = sorted(range(NPAIR), key=lambda k: (max(PAIRS[k]), min(PAIRS[k])))
    kdec = [k for k in ksort if pos[k] in (0, 4)]
    krelu = [k for k in ksort if pos[k] in (2, 3)][:NRELU]
    kcmb = [k for k in ksort if k not in kdec and k not in krelu]
    # dec pairs first (cheap consumers -> compute starts early); weave the
    # PE-heavy relu pairs with the DVE-heavy combo pairs; dec tail
    mid = []
    a, b = list(krelu), list(kcmb)
    while a or b:
        if a:
            mid.append(a.pop(0))
        if b:
            mid.append(b.pop(0))
    if len(kdec) > 2:
        korder = kdec[:-2] + mid + kdec[-2:]
    else:
        korder = kdec + mid
    return korder, set(kdec), set(krelu)


def _build_program(pos):
    korder, kdec, krelu = _plan(pos)
    krelu_idx = {k: r for r, k in enumerate(sorted(krelu))}
    nrelu = max(1, len(krelu))

    nc = bacc.Bacc("TRN2", target_bir_lowering=False, debug=False)

    nzdt = FP8 if NZ_FP8 else BF16
    nz = nc.dram_tensor("nz", [D, NPAIR * 2 * BS], nzdt, kind="ExternalInput")
    ohf = [
        nc.dram_tensor(f"ohf{h}", [D, BS], BF16, kind="ExternalInput")
        for h in range(3)
    ]
    tabf = [
        nc.dram_tensor(f"tabf{h}", [D, 2 * D], BF16, kind="ExternalInput")
        for h in range(3)
    ]
    oh96 = nc.dram_tensor("oh96", [COLS * NUM_EMB, OHW], BF16, kind="ExternalInput")
    wbf = nc.dram_tensor("wbf", [D, NPAIR * 4], BF16, kind="ExternalInput")
    ident = nc.dram_tensor("ident", [D, 2 * D], BF16, kind="ExternalInput")
    mdk = nc.dram_tensor("mdk", [COLS * NUM_EMB, nrelu * D], BF16, kind="ExternalInput")
    out = nc.dram_tensor("out", [2, BS], FP32, kind="ExternalOutput")

    n_ring = (NPAIR + NZC - 1) // NZC

    with tile.TileContext(nc) as tc:
        with (
            tc.tile_pool(name="const", bufs=1) as const_pool,
            tc.tile_pool(name="nzp", bufs=6) as nz_pool,
            tc.tile_pool(name="ms", bufs=1) as ms_pool,
            tc.tile_pool(name="tmp", bufs=2) as tmp_pool,
            tc.tile_pool(name="gpsum", bufs=3, space="PSUM") as gath_psum,
            tc.tile_pool(name="dpsum", bufs=2, space="PSUM") as diff_psum,
            tc.tile_pool(name="opsum", bufs=1, space="PSUM") as out_psum,
            tc.tile_pool(name="osb", bufs=1) as out_sb_pool,
        ):
            # --- const DMAs: warmup deps first, CM last ---
            id_sb = const_pool.tile([D, 2 * D], BF16, tag="ident")
            nc.sync.dma_start(out=id_sb[:], in_=ident[:])
            wbf_sb = const_pool.tile([D, NPAIR * 4], BF16, tag="wbf")
            nc.sync.dma_start(out=wbf_sb[:], in_=wbf[:])
            tab_sb, oh_sbt = [], []
            for h in range(3):
                tsb = const_pool.tile([D, 2 * D], BF16, tag=f"tab{h}", name=f"tsb{h}")
                nc.sync.dma_start(out=tsb[:], in_=tabf[h][:])
                tab_sb.append(tsb)
            for h in range(3):
                osb_h = const_pool.tile([D, BS], BF16, tag=f"ohf{h}", name=f"osb_h{h}")
                nc.gpsimd.dma_start(out=osb_h[:], in_=ohf[h][:])
                oh_sbt.append(osb_h)
            mdk_sb = const_pool.tile([COLS * NUM_EMB, nrelu * D], BF16, tag="mdk")
            nc.gpsimd.dma_start(out=mdk_sb[:], in_=mdk[:])
            oh96_sb = const_pool.tile([COLS * NUM_EMB, OHW], BF16, tag="oh96")
            nc.gpsimd.dma_start(out=oh96_sb[:], in_=oh96[:])
            ipos = id_sb[:, 0:D]
            ineg = id_sb[:, D : 2 * D]

            def _strip(c):
                return tab_sb[c // 3], oh_sbt[c // 3], 32 * (c % 3)

            s01_sb, mhi_sb, oh_sb = [], [], []
            for c in range(COLS):
                t, o, b = _strip(c)
                s01_sb.append(t[b : b + NUM_EMB, 0:D])
                mhi_sb.append(t[b : b + NUM_EMB, D : 2 * D])
                oh_sb.append(o[b : b + NUM_EMB, :])
            cmhi_sb = oh96_sb[:, BS : BS + 2]
            cmlo_sb = oh96_sb[:, BS + 2 : BS + 4]
            wpart = [
                (wbf_sb[:, k * 4 : k * 4 + 2], wbf_sb[:, k * 4 + 2 : k * 4 + 4])
                for k in range(NPAIR)
            ]

            # --- noise ring: casting DMAs, NZC pairs per tile.  Two tiny
            # Pool-queue reads of the const tiles first: the ring DMAs sit
            # behind them in the in-order Pool queue, so the const DMAs get
            # the DMA engines to themselves for the first few us ---
            ring = []
            for rix in range(n_ring):
                lo = rix * NZC * 2 * BS
                hi = min((rix + 1) * NZC, NPAIR) * 2 * BS
                rt = nz_pool.tile([D, hi - lo], BF16, tag="nzr", name="nzr")
                if NZ_FP8:
                    nc.gpsimd.dma_start(out=rt[:], in_=nz[:, lo:hi])
                else:
                    nc.sync.dma_start(out=rt[:], in_=nz[:, lo:hi])
                ring.append(rt)

            # --- PE warm-up (~3.4us of junk burns the cold window) ---
            junk = gath_psum.tile([D, 2 * D], FP32, tag="junk", name="junk", bufs=1)
            for wi in range(NWARM):
                nc.tensor.matmul(junk[:], ipos, id_sb[:], start=True, stop=True)


            # --- gathers: s for all cols, m for mul/plain-combo cols,
            # into two big tiles (block per column) for fused-AP reads ---
            s_cols, m_cols = [], []
            for k in korder:
                for c in PAIRS[k]:
                    if c not in s_cols:
                        s_cols.append(c)
                    if k not in kdec and k not in krelu and c not in m_cols:
                        m_cols.append(c)

            s_big = ms_pool.tile([D, COLS * BS], BF16, tag="sbig", name="s_big")
            m_big = ms_pool.tile([D, COLS * BS], BF16, tag="mbig", name="m_big")
            s_view = s_big[:].rearrange("p (c b) -> p c b", c=COLS)
            m_view = m_big[:].rearrange("p (c b) -> p c b", c=COLS)
            for c in s_cols:
                for ch in range(NCH):
                    g2 = gath_psum.tile([D, CH], FP32, tag="g", name="g")
                    nc.tensor.matmul(
                        g2[:], s01_sb[c], oh_sb[c][:, bass.ts(ch, CH)],
                        start=True, stop=True,
                    )
                    nc.scalar.copy(s_big[:, c * BS + ch * CH : c * BS + (ch + 1) * CH], g2[:])
            for c in m_cols:
                for ch in range(NCH):
                    g = gath_psum.tile([D, CH], FP32, tag="g", name="g")
                    nc.tensor.matmul(
                        g[:], mhi_sb[c], oh_sb[c][:, bass.ts(ch, CH)],
                        start=True, stop=True,
                    )
                    nc.scalar.copy(m_big[:, c * BS + ch * CH : c * BS + (ch + 1) * CH], g[:])

            # bridge: dep-free PE work covering the gather->pair handoff
            for wi in range(8):
                nc.tensor.matmul(junk[:], ipos, id_sb[:], start=True, stop=True)

            # --- output accumulators + matmul budget per chunk ---
            acc = [
                out_psum.tile([2, CH], FP32, tag=f"acc{ch}", name=f"acc{ch}")
                for ch in range(NCH)
            ]
            n_mm = [2] * NCH  # CM hi+lo
            for k in range(NPAIR):
                per = 1 if (k not in kdec and k not in krelu) else 2
                for ch in range(NCH):
                    n_mm[ch] += per
            done_mm = [0] * NCH

            def acc_mm(ch, lhsT, rhs):
                done_mm[ch] += 1
                nc.tensor.matmul(
                    acc[ch][:], lhsT, rhs,
                    start=(done_mm[ch] == 1),
                    stop=(done_mm[ch] == n_mm[ch]),
                )

            # --- pair loop with LAG-deferred accumulation (PE reservoir) ---
            pend = deque()

            def emit_acc(k, rhs0, rhs1):
                for ch in range(NCH):
                    acc_mm(ch, wpart[k][0], rhs0[:, bass.ts(ch, CH)])
                if rhs1 is not None:
                    for ch in range(NCH):
                        acc_mm(ch, wpart[k][1], rhs1[:, bass.ts(ch, CH)])

            for jk, k in enumerate(korder):
                # flush one deferred acc first: ready PE work ahead of stalls
                if len(pend) > LAG:
                    emit_acc(*pend.popleft())

                i, j = PAIRS[k]
                l = pos[k]
                rt = ring[jk // NZC]
                slot = (jk % NZC) * 2 * BS
                n01 = rt[:, slot : slot + 2 * BS]

                t = tmp_pool.tile([D, 2 * BS], BF16, tag="t", name="t", bufs=LAG + 3)
                t0 = t[:, 0:BS]
                t1 = t[:, BS : 2 * BS]
                # fused two-block multiply: t = s[(i,j) blocks] * n01
                nc.vector.tensor_tensor(
                    t[:].rearrange("p (s b) -> p s b", s=2),
                    s_view[:, i : j + 1 : (j - i)],
                    n01.rearrange("p (s b) -> p s b", s=2),
                    mybir.AluOpType.mult,
                )

                if k in kdec:
                    pend.append((k, t0, t1))
                elif k in krelu:
                    ridx = krelu_idx[k]
                    md_l = mdk_sb[:, ridx * D : (ridx + 1) * D]
                    r = tmp_pool.tile([D, BS], BF16, tag="r", name="r", bufs=LAG + 2)
                    dpss = []
                    for ch in range(NCH):
                        dps = diff_psum.tile([D, CH], FP32, tag="d", name="d")
                        nc.tensor.matmul(
                            dps[:], md_l, oh96_sb[:, bass.ts(ch, CH)],
                            start=True, stop=False,
                        )
                        dpss.append(dps)
                    if DN_DVE:
                        dn = tmp_pool.tile([D, BS], BF16, tag="dn", name="dn", bufs=2)
                        if l == 2:
                            nc.vector.tensor_tensor(dn[:], t1, t0, mybir.AluOpType.subtract)
                        else:
                            nc.vector.tensor_tensor(dn[:], t0, t1, mybir.AluOpType.subtract)
                        for ch in range(NCH):
                            nc.tensor.matmul(
                                dpss[ch][:], ipos, dn[:, bass.ts(ch, CH)],
                                start=False, stop=True,
                            )
                    else:
                        ta, tb = (t1, t0) if l == 2 else (t0, t1)
                        for ch in range(NCH):
                            nc.tensor.matmul(
                                dpss[ch][:], ipos, ta[:, bass.ts(ch, CH)],
                                start=False, stop=False,
                            )
                            nc.tensor.matmul(
                                dpss[ch][:], ineg, tb[:, bass.ts(ch, CH)],
                                start=False, stop=True,
                            )
                    for ch in range(NCH):
                        nc.scalar.activation(
                            r[:, bass.ts(ch, CH)], dpss[ch][:],
                            mybir.ActivationFunctionType.Relu,
                        )
                    pend.append((k, t0, r[:]))
                else:
                    # mul pair: fused add p|q = t + m[(i,j) blocks], then p*q
                    pq = tmp_pool.tile([D, 2 * BS], BF16, tag="pq", name="pq", bufs=3)
                    nc.vector.tensor_tensor(
                        pq[:].rearrange("p (s b) -> p s b", s=2),
                        t[:].rearrange("p (s b) -> p s b", s=2),
                        m_view[:, i : j + 1 : (j - i)],
                        mybir.AluOpType.add,
                    )
                    combo = tmp_pool.tile([D, BS], BF16, tag="cb", name="cb", bufs=3)
                    nc.vector.tensor_tensor(
                        combo[:], pq[:, 0:BS], pq[:, BS : 2 * BS], _ALU[l]
                    )
                    pend.append((k, combo[:], None))

            while pend:
                emit_acc(*pend.popleft())

            # mean path (decomposed + relu pairs): stacked K=96, hi+lo
            for ch in range(NCH):
                acc_mm(ch, cmhi_sb, oh96_sb[:, bass.ts(ch, CH)])
                acc_mm(ch, cmlo_sb, oh96_sb[:, bass.ts(ch, CH)])

            # --- write out ---
            osb = out_sb_pool.tile([2, BS], FP32, tag="osb", name="osb")
            for ch in range(NCH):
                nc.scalar.copy(osb[:, bass.ts(ch, CH)], acc[ch][:])
            nc.sync.dma_start(out=out[:], in_=osb[:])

    return nc


def _prepare_inputs(features, emb_mean, emb_std, W_nc, W_cat, log_alpha, noise):
    features = np.asarray(features)
    emb_mean = np.ascontiguousarray(np.asarray(emb_mean, dtype=np.float32))
    emb_std = np.asarray(emb_std, dtype=np.float32)
    W_nc = np.asarray(W_nc, dtype=np.float32)
    W_cat = np.asarray(W_cat, dtype=np.float32)
    log_alpha = np.asarray(log_alpha, dtype=np.float32)
    noise = np.asarray(noise, dtype=np.float32)

    pos = np.argmax(log_alpha, axis=-1).tolist()
    korder, kdec, krelu = _plan(pos)

    s01 = np.logaddexp(0.0, emb_std).astype(np.float32) * np.float32(0.01)

    onehot = (
        features[:, None, :] == np.arange(NUM_EMB, dtype=features.dtype)[None, :, None]
    ).astype(np.float32)

    # per-pair selected weights as lhsT [D, 2] x 2 parts
    wparts = np.zeros((NPAIR, 2, D, 2), dtype=np.float32)
    for k in range(NPAIR):
        l = pos[k]
        if l == 4:
            wparts[k, 0] = W_cat[k, :, :D].T
            wparts[k, 1] = W_cat[k, :, D:].T
        elif k in krelu:
            w = W_nc[k, l].T
            wparts[k, 0] = w
            wparts[k, 1] = w if l == 2 else -w  # min: out = p@W - relu@W
        else:
            wparts[k, 0] = W_nc[k, l].T
            wparts[k, 1] = W_nc[k, l].T

    wbf = np.zeros((D, NPAIR * 4), dtype=BF)
    cm = np.zeros((COLS, NUM_EMB, 2), dtype=np.float32)
    for k in range(NPAIR):
        i, j = PAIRS[k]
        for pi in range(2):
            wbf[:, k * 4 + 2 * pi : k * 4 + 2 * pi + 2] = wparts[k, pi].astype(BF)
        if k in kdec:
            cm[i] += emb_mean[i] @ wparts[k, 0]
            cm[j] += emb_mean[j] @ wparts[k, 1]
        elif k in krelu:
            cm[i] += emb_mean[i] @ W_nc[k, pos[k]].T  # p-mean (base side i)

    tab_ab = np.zeros((3, D, 2 * D), dtype=BF)
    for c in range(COLS):
        b = 32 * (c % 3)
        tab_ab[c // 3, b : b + NUM_EMB, 0:D] = s01[c].astype(BF)
        tab_ab[c // 3, b : b + NUM_EMB, D : 2 * D] = emb_mean[c].astype(BF)

    cm_hi = cm.astype(BF)
    cm_lo = (cm - cm_hi.astype(np.float32)).astype(BF)
    oh96_base = np.zeros((COLS * NUM_EMB, OHW), dtype=BF)
    oh96_base[:, BS : BS + 2] = cm_hi.reshape(COLS * NUM_EMB, 2)
    oh96_base[:, BS + 2 : BS + 4] = cm_lo.reshape(COLS * NUM_EMB, 2)

    ident = np.zeros((D, 2 * D), dtype=BF)
    ident[:, 0:D] = np.eye(D, dtype=np.float32)
    ident[:, D : 2 * D] = -np.eye(D, dtype=np.float32)

    # stacked +-M diff tables for relu pairs: [96, nrelu*D]
    krelu_s = sorted(krelu)
    nrelu = max(1, len(krelu_s))
    mdkt = np.zeros((COLS * NUM_EMB, nrelu * D), dtype=BF)
    for ridx, k in enumerate(krelu_s):
        i, j = PAIRS[k]
        blk = np.zeros((COLS * NUM_EMB, D), dtype=np.float32)
        if pos[k] == 2:  # max: + Mtab_j, - Mtab_i
            blk[j * NUM_EMB : (j + 1) * NUM_EMB] = emb_mean[j]
            blk[i * NUM_EMB : (i + 1) * NUM_EMB] = -emb_mean[i]
        else:  # min: + Mtab_i, - Mtab_j
            blk[i * NUM_EMB : (i + 1) * NUM_EMB] = emb_mean[i]
            blk[j * NUM_EMB : (j + 1) * NUM_EMB] = -emb_mean[j]
        mdkt[:, ridx * D : (ridx + 1) * D] = blk.astype(BF)

    # noise: [NPAIR, 2, B, D] -> per-core [D, NPAIR*2*BS] in korder
    nzdt = F8 if NZ_FP8 else BF
    nk = noise[korder]

    in_maps = []
    for cidx in range(NCORES):
        sl = slice(cidx * BS, (cidx + 1) * BS)
        nz_core = np.ascontiguousarray(
            nk[:, :, sl, :].transpose(3, 0, 1, 2).reshape(D, NPAIR * 2 * BS)
        ).astype(nzdt)
        oh_arr = oh96_base.copy()
        ohab = np.zeros((3, D, BS), dtype=BF)
        for col in range(COLS):
            oh_arr[col * NUM_EMB : (col + 1) * NUM_EMB, :BS] = onehot[col][:, sl]
            ohab[col // 3, 32 * (col % 3) : 32 * (col % 3) + NUM_EMB, :] = onehot[col][:, sl]
        in_maps.append(
            {
                "nz": nz_core,
                "ohf0": ohab[0],
                "ohf1": ohab[1],
                "ohf2": ohab[2],
                "tabf0": tab_ab[0],
                "tabf1": tab_ab[1],
                "tabf2": tab_ab[2],
                "oh96": oh_arr,
                "wbf": wbf,
                "ident": ident,
                "mdk": mdkt,
            }
        )
    return pos, in_maps


def _run(inputs: dict, trace: bool = False):
    pos, in_maps = _prepare_inputs(**inputs)
    nc = _build_program(pos)
    nc.finalize()
    res = run_bass_kernel_spmd(nc, in_maps, list(range(NCORES)), trace=trace)
    out = np.empty((B, 2), dtype=np.float32)
    for c in range(NCORES):
        out[c * BS : (c + 1) * BS, :] = res.results[c]["out"].T
    return out, res


def kernel(**inputs) -> np.ndarray:
    out, _ = _run(inputs, trace=False)
    return out
